# revision 11
# baseline (speedup 1.0000x reference)
"""AIMv2 multi-head attention (B=4, N=2048, C=1024, H=8) on 8 TRN2 NeuronCores.

Sharding: (batch x token-half) -> 8 shards, one per core. Each core:
  - computes q for its 1024 query tokens (all heads),
  - redundantly computes k / v^T for its batch's full 2048 tokens
    (cheaper than any cross-core collective at these sizes),
  - does softmax(q k^T / sqrt(D)) v for all 8 heads with scores built
    TRANSPOSED ([k_t, q_t]) so the PE contracts over the partition dim in
    both matmuls without any on-chip transposes,
  - applies the output projection and writes its [1024, 1024] slice.
No collectives. Weights/x are pre-transposed and cast to bf16 on the host
so every matmul operand is in its natural [K(part), M/N(free)] layout.
"""

import sys

sys.path.insert(0, "/opt/trn_rl_repo")

import numpy as np
import ml_dtypes

import concourse.bass as bass
import concourse.mybir as mybir
import concourse.tile as tile
from concourse import bacc
from concourse.bass_utils import run_bass_kernel_spmd

B, N, C, H, D = 4, 2048, 1024, 8, 128
NQ = N // 2          # query tokens per core
CB = C // 128        # contraction blocks
KT = N // 128        # key-token 128-blocks
BF = mybir.dt.bfloat16
F32 = mybir.dt.float32
SCALE = float(1.0 / np.sqrt(D))
AF = mybir.ActivationFunctionType
ALU = mybir.AluOpType


def _emit(nc, tc, pools, params, r):
    """Emit one full forward pass. r = rep index (benchmarking only)."""
    wp, xp, qkvp, ppool, misc, psA, psB, psO = pools
    xT, xqT, WqkvT, WprojT, out = params
    ones, ones_row = pools.ones, pools.ones_row

    q_sb = [qkvp.tile([128, NQ], BF, tag=f"q{h}", name=f"r{r}q{h}") for h in range(H)]
    k_sb = [qkvp.tile([128, N], BF, tag=f"k{h}", name=f"r{r}k{h}") for h in range(H)]
    v_sb = [qkvp.tile([128, C], BF, tag=f"v{kt}", name=f"r{r}v{kt}") for kt in range(KT)]
    a_sb = [qkvp.tile([128, NQ], BF, tag=f"a{h}", name=f"r{r}a{h}") for h in range(H)]

    # ---- QKV projections -------------------------------------------------
    # q pass: q[o, t] for own-half tokens; W stationary, x moving
    wq = [wp.tile([128, C], BF, tag=f"w{c}", name=f"r{r}wq{c}") for c in range(CB)]
    for c in range(CB):
        nc.sync.dma_start(out=wq[c], in_=WqkvT[c * 128:(c + 1) * 128, 0:C])
    for t2 in range(NQ // 512):
        xt = [xp.tile([128, 512], BF, tag=f"x{c}", name=f"r{r}xq{c}_{t2}") for c in range(CB)]
        for c in range(CB):
            nc.sync.dma_start(out=xt[c], in_=xqT[c * 128:(c + 1) * 128, t2 * 512:(t2 + 1) * 512])
        for h in range(H):
            ps = psB.tile([128, 512], F32, tag="b", name=f"r{r}psq{t2}_{h}")
            for c in range(CB):
                nc.tensor.matmul(ps, lhsT=wq[c][:, h * 128:(h + 1) * 128], rhs=xt[c],
                                 start=(c == 0), stop=(c == CB - 1))
            nc.scalar.copy(out=q_sb[h][:, t2 * 512:(t2 + 1) * 512], in_=ps)

    # k pass: k[o, t] for all 2048 tokens
    wk = [wp.tile([128, C], BF, tag=f"w{c}", name=f"r{r}wk{c}") for c in range(CB)]
    for c in range(CB):
        nc.sync.dma_start(out=wk[c], in_=WqkvT[c * 128:(c + 1) * 128, C:2 * C])
    for t2 in range(N // 512):
        xt = [xp.tile([128, 512], BF, tag=f"x{c}", name=f"r{r}xk{c}_{t2}") for c in range(CB)]
        for c in range(CB):
            nc.sync.dma_start(out=xt[c], in_=xT[c * 128:(c + 1) * 128, t2 * 512:(t2 + 1) * 512])
        for h in range(H):
            ps = psB.tile([128, 512], F32, tag="b", name=f"r{r}psk{t2}_{h}")
            for c in range(CB):
                nc.tensor.matmul(ps, lhsT=wk[c][:, h * 128:(h + 1) * 128], rhs=xt[c],
                                 start=(c == 0), stop=(c == CB - 1))
            nc.vector.tensor_copy(out=k_sb[h][:, t2 * 512:(t2 + 1) * 512], in_=ps)

    # v pass: vT[t, o] for all tokens; x stationary, W moving
    wv = [wp.tile([128, C], BF, tag=f"w{c}", name=f"r{r}wv{c}") for c in range(CB)]
    for c in range(CB):
        nc.sync.dma_start(out=wv[c], in_=WqkvT[c * 128:(c + 1) * 128, 2 * C:3 * C])
    for t2 in range(N // 512):
        xt = [xp.tile([128, 512], BF, tag=f"x{c}", name=f"r{r}xv{c}_{t2}") for c in range(CB)]
        for c in range(CB):
            nc.sync.dma_start(out=xt[c], in_=xT[c * 128:(c + 1) * 128, t2 * 512:(t2 + 1) * 512])
        for s4 in range(4):
            kt = t2 * 4 + s4
            for o2 in range(2):
                ps = psB.tile([128, 512], F32, tag="b", name=f"r{r}psv{kt}_{o2}")
                for c in range(CB):
                    nc.tensor.matmul(ps, lhsT=xt[c][:, s4 * 128:(s4 + 1) * 128],
                                     rhs=wv[c][:, o2 * 512:(o2 + 1) * 512],
                                     start=(c == 0), stop=(c == CB - 1))
                nc.scalar.copy(out=v_sb[kt][:, o2 * 512:(o2 + 1) * 512], in_=ps)

    # ---- attention (per head, scores transposed: S^T[k_t, q_t]) ---------
    for h in range(H):
        pts = [ppool.tile([128, NQ], BF, tag=f"p{kt}", name=f"r{r}p{h}_{kt}") for kt in range(KT)]
        for kt in range(KT):
            sps = psA.tile([128, NQ], F32, tag="s", name=f"r{r}s{h}_{kt}")
            for q2 in range(NQ // 512):
                nc.tensor.matmul(sps[:, q2 * 512:(q2 + 1) * 512],
                                 lhsT=k_sb[h][:, kt * 128:(kt + 1) * 128],
                                 rhs=q_sb[h][:, q2 * 512:(q2 + 1) * 512],
                                 start=True, stop=True)
            nc.scalar.activation(pts[kt], sps, AF.Exp, scale=SCALE)

        # denominator: DVE-accumulate P^T over key blocks, then a ones-row
        # matmul folds the remaining 128 partitions
        acc = misc.tile([128, NQ], F32, tag="acc", bufs=1, name=f"r{r}acc{h}")
        nc.vector.tensor_tensor(out=acc, in0=pts[0], in1=pts[1], op=ALU.add)
        for kt in range(2, KT):
            nc.vector.tensor_tensor(out=acc, in0=acc, in1=pts[kt], op=ALU.add)
        recip = misc.tile([1, NQ], F32, tag="recip", bufs=1, name=f"r{r}recip{h}")
        for q2 in range(NQ // 512):
            den = psB.tile([128, 512], F32, tag="b", name=f"r{r}den{h}_{q2}")
            nc.tensor.matmul(den[0:1, :], lhsT=ones, rhs=acc[:, q2 * 512:(q2 + 1) * 512],
                             start=True, stop=True)
            nc.vector.reciprocal(recip[:, q2 * 512:(q2 + 1) * 512], den[0:1, :])

        ops = psO.tile([128, NQ], F32, tag="o", name=f"r{r}ov{h}")
        for q2 in range(NQ // 512):
            for kt in range(KT):
                nc.tensor.matmul(ops[:, q2 * 512:(q2 + 1) * 512],
                                 lhsT=v_sb[kt][:, h * 128:(h + 1) * 128],
                                 rhs=pts[kt][:, q2 * 512:(q2 + 1) * 512],
                                 start=(kt == 0), stop=(kt == KT - 1))
        # broadcast recip across partitions via outer product, then scale
        for q2 in range(NQ // 512):
            bc = psB.tile([128, 512], F32, tag="b", name=f"r{r}bc{h}_{q2}")
            nc.tensor.matmul(bc, lhsT=ones_row, rhs=recip[:, q2 * 512:(q2 + 1) * 512],
                             start=True, stop=True)
            bcs = misc.tile([128, 512], F32, tag="bcs", name=f"r{r}bcs{h}_{q2}")
            nc.scalar.copy(out=bcs, in_=bc)
            nc.vector.tensor_tensor(out=a_sb[h][:, q2 * 512:(q2 + 1) * 512],
                                    in0=ops[:, q2 * 512:(q2 + 1) * 512],
                                    in1=bcs, op=ALU.mult)

    # ---- output projection ----------------------------------------------
    wpj = [wp.tile([128, C], BF, tag=f"w{c}", name=f"r{r}wpj{c}") for c in range(CB)]
    for c in range(CB):
        nc.sync.dma_start(out=wpj[c], in_=WprojT[c * 128:(c + 1) * 128, :])
    for t in range(NQ // 128):
        for o2 in range(2):
            ps = psB.tile([128, 512], F32, tag="b", name=f"r{r}pp{t}_{o2}")
            for c in range(CB):
                nc.tensor.matmul(ps, lhsT=a_sb[c][:, t * 128:(t + 1) * 128],
                                 rhs=wpj[c][:, o2 * 512:(o2 + 1) * 512],
                                 start=(c == 0), stop=(c == CB - 1))
            stg = misc.tile([128, 512], F32, tag="ostg", name=f"r{r}stg{t}_{o2}")
            nc.scalar.copy(out=stg, in_=ps)
            nc.sync.dma_start(out=out[t * 128:(t + 1) * 128, o2 * 512:(o2 + 1) * 512], in_=stg)


class _Pools(tuple):
    pass


def build_bass(reps: int = 1) -> bass.Bass:
    nc = bacc.Bacc("TRN2", target_bir_lowering=False, debug=False, num_devices=8)
    xT = nc.declare_dram_parameter("xT", [C, N], BF, isOutput=False)
    xqT = nc.declare_dram_parameter("xqT", [C, NQ], BF, isOutput=False)
    WqkvT = nc.declare_dram_parameter("WqkvT", [C, 3 * C], BF, isOutput=False)
    WprojT = nc.declare_dram_parameter("WprojT", [C, C], BF, isOutput=False)
    out = nc.declare_dram_parameter("out", [NQ, C], F32, isOutput=True)
    params = (xT, xqT, WqkvT, WprojT, out)

    with tile.TileContext(nc) as tc:
        with (
            tc.tile_pool(name="wp", bufs=2) as wp,        # pass weights
            tc.tile_pool(name="xp", bufs=2) as xp,        # streamed x tiles
            tc.tile_pool(name="qkv", bufs=1) as qkvp,     # resident q/k/vT/attn
            tc.tile_pool(name="pp", bufs=1) as ppool,     # exp'd probs P^T
            tc.tile_pool(name="misc", bufs=2) as misc,
            tc.tile_pool(name="psA", bufs=2, space="PSUM") as psA,  # scores [128,1024]
            tc.tile_pool(name="psB", bufs=2, space="PSUM") as psB,  # [128,512] qkv/proj/den
            tc.tile_pool(name="psO", bufs=1, space="PSUM") as psO,  # PV out [128,1024]
        ):
            pools = _Pools((wp, xp, qkvp, ppool, misc, psA, psB, psO))
            ones = misc.tile([128, 1], F32, tag="ones", bufs=1, name="ones")
            nc.vector.memset(ones, 1.0)
            ones_row = misc.tile([1, 128], F32, tag="ones_row", bufs=1, name="ones_row")
            nc.vector.memset(ones_row, 1.0)
            pools.ones, pools.ones_row = ones, ones_row
            for r in range(reps):
                _emit(nc, tc, pools, params, r)
    nc.compile()
    return nc


_NC_CACHE = {}


def _get_nc(reps: int = 1):
    if reps not in _NC_CACHE:
        _NC_CACHE[reps] = build_bass(reps)
    return _NC_CACHE[reps]


def _make_in_maps(x, Wqkv, Wproj):
    bf = ml_dtypes.bfloat16
    WqkvT = np.ascontiguousarray(Wqkv.T).astype(bf)
    WprojT = np.ascontiguousarray(Wproj.T).astype(bf)
    in_maps = []
    for core in range(8):
        b, half = core // 2, core % 2
        xT_b = np.ascontiguousarray(x[b].T).astype(bf)
        xqT = np.ascontiguousarray(x[b, half * NQ:(half + 1) * NQ].T).astype(bf)
        in_maps.append({"xT": xT_b, "xqT": xqT, "WqkvT": WqkvT, "WprojT": WprojT})
    return in_maps


def _assemble(results):
    out = np.empty((B, N, C), np.float32)
    for core in range(8):
        b, half = core // 2, core % 2
        out[b, half * NQ:(half + 1) * NQ] = results[core]["out"]
    return out


def run_reps(x, Wqkv, Wproj, reps: int = 1):
    """Benchmarking entry: same kernel body emitted `reps` times in one NEFF."""
    res = run_bass_kernel_spmd(_get_nc(reps), _make_in_maps(x, Wqkv, Wproj),
                               core_ids=list(range(8)))
    return _assemble(res.results), res


def kernel(x, Wqkv, Wproj):
    res = run_bass_kernel_spmd(_get_nc(1), _make_in_maps(x, Wqkv, Wproj),
                               core_ids=list(range(8)))
    return _assemble(res.results)


if __name__ == "__main__":
    nc = build_bass()
    print("built ok")
